# revision 32
# baseline (speedup 1.0000x reference)
"""Trainium2 Bass kernel for nn_CrossAttentionFusion.

Reference computation (B=16384, img_dim=2048, tab_dim=128, E=256):
    img_p   = img_embed @ Wi.T + bi                      (B, E)
    tab_p   = tab_embed @ Wt.T + bt                      (B, E)
    img_att = LN(tab_p @ Wc_img.T + bc_img + img_p)      Wc_img = out_w_img @ Wv_img
    tab_att = LN(img_p @ Wc_tab.T + bc_tab + tab_p)
    out     = concat([img_att, tab_att], -1)             (B, 2E)

Host-side algebra (exact):
  * The two 256x256 attention matmuls fold into one:  Wc = out_w @ in_w[2E:].
  * The img-attention path folds all the way to tab_embed:
        tab_p @ Wc_img.T = tab_embed @ Wfold_img.T,  Wfold_img = Wc_img @ Wt
  * s_img = img_p + img_att_pre accumulates IN PSUM (both are matmuls into
    the same bank), so no DVE residual add is needed.
  * The tab path needs raw img_p but PSUM holds s_img; transpose s_img
    instead and correct with  Wt_eff = Wt - Wc_tab @ Wfold_img:
        s_tab = xt @ Wt_eff.T + s_img @ Wc_tab.T        (exact)
    so s_tab also accumulates fully in PSUM.
  * All biases collapse into one per-side vector added before LN.

Device plan (pure data parallel, batch sharded 8 ways, weights replicated);
per 128-row b-tile jj, 4-stage software pipeline:
    S1(jj):   PE  pB  = xt @ Wt_eff.T          (1 MM, start)
              PE  pA  = sum_k xi_k @ Wi_k.T    (16 MMs) + xt @ Wfold_img.T
                                               (1 MM, stop)  == s_img
    S2(jj-1): ACT sA  = bf16(pA);  DVE bn_stats/aggr(pA)
    S3(jj-2): PE  pT  = transpose(sA);  POOL sT = bf16(pT)
    S4(jj-3): PE  pB += sT.T @ Wc_tab.T        (2 MMs, stop)  == s_tab
              DVE bn_stats/aggr(pB); ACT sqrt(var+eps); DVE rstd
              DVE apply (s-mean)*rstd -> o (bf16); store o per b-tile
LN stats/apply read PSUM directly; the only SBUF staging is the bf16
transpose path and the output tile.  Engines balance roughly as
PE 36us / DMA 34us / DVE 20us / ACT 7us / POOL 9us per core.
"""

import json
import os

import numpy as np

E = 256
IMG_DIM = 2048
TAB_DIM = 128
B_FULL = 16384
N_CORES = 8
B_LOC = B_FULL // N_CORES  # 2048
P = 128
KT = IMG_DIM // P  # 16 k-tiles for the img contraction
EPS = 1e-5

# "bf16" (bf16 HBM data + matmuls + bf16 output staging, rel err ~2.5e-3)
# or "f32r" (fp32 data, fp32r matmuls, rel err ~1.5e-4, ~1.7x slower)
MM_MODE = os.environ.get("KERNEL_MM_MODE", "bf16")

_cache: dict = {}


def _split_multi_waits(bir_bytes: bytes) -> bytes:
    """Work around this walrus build's 1-sync-wait-per-instruction limit.

    Any BIR instruction with >1 `on_wait` fails codegen ("Too many sync wait
    commands").  Hoist all but the last wait onto same-engine EventSemaphore
    instructions inserted immediately before; engines run their stream in
    order, so sequential sem waits are equivalent.
    """
    m = json.loads(bir_bytes)
    for f in m["functions"]:
        for b in f["blocks"]:
            out = []
            for ins in b["instructions"]:
                si = ins.get("sync_info")
                waits = (si or {}).get("on_wait") or []
                if len(waits) > 1:
                    for i, extra in enumerate(waits[:-1]):
                        out.append(
                            {
                                "debug": ins.get("debug", 0),
                                "engine": ins["engine"],
                                "ins": [],
                                "outs": [],
                                "name": f"{ins['name']}-ws{i}",
                                "opcode": "EventSemaphore",
                                "sync_info": {"on_update": [], "on_wait": [extra]},
                            }
                        )
                    si["on_wait"] = [waits[-1]]
                out.append(ins)
            b["instructions"] = out
    return json.dumps(m).encode()


def _build_module(use_bias: bool, use_gb: bool, mode: str, reps: int = 1,
                  unroll: int = 1):
    """reps>1 wraps the body in a hardware loop (benchmarking only);
    unroll>1 emits the body N times statically (TimelineSim can't follow
    hardware loops, so steady-state sims use unroll instead)."""
    import contextlib

    import concourse.bass as bass
    import concourse.mybir as mybir
    import concourse.tile as tile
    from concourse.masks import make_identity

    f32 = mybir.dt.float32
    xdt = {
        "f32r": mybir.dt.float32r,
        "bf16": mybir.dt.bfloat16,
        "f32": f32,
    }[mode]
    # single-dtype PE stream (transposes in the matmul dtype): mixing fp32
    # 2-pass transposes with f32r matmuls intermittently faulted HW
    tdt = xdt if mode != "f32" else f32
    odt = mybir.dt.bfloat16 if mode == "bf16" else f32

    nc = bass.Bass()

    xiT = nc.dram_tensor("xiT", [IMG_DIM, B_LOC], xdt, kind="ExternalInput")
    xtT = nc.dram_tensor("xtT", [TAB_DIM, B_LOC], xdt, kind="ExternalInput")
    wiT = nc.dram_tensor("wiT", [IMG_DIM, E], xdt, kind="ExternalInput")
    # [Wfold_img.T | Wt_eff.T]  (TAB_DIM, 2E)
    wxt = nc.dram_tensor("wxt", [TAB_DIM, 2 * E], xdt, kind="ExternalInput")
    wctT = nc.dram_tensor("wctT", [E, E], xdt, kind="ExternalInput")
    out = nc.dram_tensor("out", [B_LOC, 2 * E], odt, kind="ExternalOutput")
    if use_bias:
        bias_d = nc.dram_tensor("bias", [2 * E], f32, kind="ExternalInput")
    if use_gb:
        lng_d = nc.dram_tensor("lng", [E], f32, kind="ExternalInput")
        lnb_d = nc.dram_tensor("lnb", [E], f32, kind="ExternalInput")

    sub = mybir.AluOpType.subtract
    mult = mybir.AluOpType.mult

    with tile.TileContext(nc) as tc:
        with (
            tc.tile_pool(name="consts", bufs=1) as consts,
            tc.tile_pool(name="xi_pool", bufs=28) as xi_pool,
            tc.tile_pool(name="xt_pool", bufs=3) as xt_pool,
            tc.tile_pool(name="work", bufs=4) as work,
            tc.tile_pool(name="outp", bufs=5) as outp,
            tc.tile_pool(name="psAB", bufs=5, space="PSUM") as psAB,
            tc.tile_pool(name="psT", bufs=3, space="PSUM") as psT,
        ):
            # ---- constants ----
            # lead-in-critical loads ride the SP ring in consumption order:
            # wxt (first matmul) -> wi_c0 (k=0..3) -> [in-loop: xt, xi...];
            # the rest of the weights follow on the ACT ring.
            KC = KT // 4
            wiT_r = wiT.rearrange("(t p) e -> p t e", p=P)
            wxt_sb = consts.tile([P, 2 * E], xdt)
            nc.sync.dma_start(out=wxt_sb, in_=wxt.ap())
            wi_cs = []
            for c in range(4):
                w = consts.tile([P, KC, E], xdt, name=f"wi_c{c}")
                ring = nc.sync if c == 0 else nc.scalar
                ring.dma_start(
                    out=w, in_=wiT_r[:, c * KC : (c + 1) * KC, :]
                )
                wi_cs.append(w)
            wct_sb = consts.tile([P, 2, E], xdt)
            nc.scalar.dma_start(out=wct_sb, in_=wctT.rearrange("(t p) e -> p t e", p=P))
            ident_f = consts.tile([P, P], f32)
            make_identity(nc, ident_f)
            if tdt == f32:
                ident = ident_f
            else:
                ident = consts.tile([P, P], tdt)
                nc.scalar.copy(ident, ident_f)
            eps_col = consts.tile([P, 1], f32)
            nc.vector.memset(eps_col, EPS)

            # PE warm-up: dummy bf16 matmuls during the DMA lead-in so the
            # HAM clock gate opens (1.2 -> 2.4 GHz) before real work.
            ident_w = consts.tile([P, P], mybir.dt.bfloat16)
            make_identity(nc, ident_w)
            warm_ps = psAB.tile([P, P], f32, name="warm_ps", tag="pAB")
            for _ in range(24):
                nc.tensor.matmul(warm_ps, lhsT=ident_w, rhs=ident_w,
                                 start=True, stop=True)
            if use_bias:
                bias_sb = consts.tile([P, 2 * E], f32)
                nc.sync.dma_start(out=bias_sb, in_=bias_d.ap().to_broadcast((P, 2 * E)))
            if use_gb:
                lng_sb = consts.tile([P, E], f32)
                nc.sync.dma_start(out=lng_sb, in_=lng_d.ap().to_broadcast((P, E)))
                lnb_sb = consts.tile([P, E], f32)
                nc.sync.dma_start(out=lnb_sb, in_=lnb_d.ap().to_broadcast((P, E)))

            xiT_r = xiT.rearrange("(t p) b -> p t b", p=P)
            out_r = out.rearrange("(t p) e -> p t e", p=P)

            # slab widths taper at the end so the final b-tile is gated by a
            # small trailing load
            SLAB_W = [512, 512, 512, 384, 128]
            assert sum(SLAB_W) == B_LOC
            slab_b0 = [sum(SLAB_W[:i]) for i in range(len(SLAB_W))]
            btiles = []
            for s, w in enumerate(SLAB_W):
                for j in range(w // P):
                    btiles.append((s, j))
            NB = len(btiles)

            # k-chunks per slab: finer for slab 0 so the first matmuls
            # release after a small transfer
            NCH = [8, 4, 4, 4, 4]

            loop_cm = tc.For_i(0, reps, 1) if reps > 1 else contextlib.nullcontext()
            with loop_cm:
              for it in range(unroll):
                chunks: dict = {}
                xts: dict = {}
                st: dict = {}

                def load_slab(s):
                    w = SLAB_W[s]
                    kc = KT // NCH[s]
                    bs = slice(slab_b0[s], slab_b0[s] + w)
                    xt = xt_pool.tile([P, w], xdt, tag="xt", name=f"xt{s}")
                    nc.sync.dma_start(out=xt, in_=xtT[:, bs])
                    xts[s] = xt
                    for c in range(NCH[s]):
                        t = xi_pool.tile([P, kc, w], xdt, tag="xi",
                                         name=f"xi{s}_{c}")
                        ks = slice(c * kc, (c + 1) * kc)
                        nc.sync.dma_start(out=t, in_=xiT_r[:, ks, bs])
                        chunks[(s, c)] = t

                def stage1(jj):
                    """all PSUM-accumulating projection matmuls for b-tile jj"""
                    s, j = btiles[jj]
                    bcol = slice(j * P, (j + 1) * P)
                    kc = KT // NCH[s]
                    # s_img in pAB[:, 0, :], s_tab in pAB[:, 1, :] — one bank
                    pAB = psAB.tile([P, 2, E], f32, tag="pAB", name=f"pAB{jj}")
                    nc.tensor.matmul(pAB, lhsT=xts[s][:, bcol], rhs=wxt_sb,
                                     start=True, stop=False)
                    pA = pAB[:, 0, :]
                    for k in range(KT):
                        nc.tensor.matmul(
                            pA,
                            lhsT=chunks[(s, k // kc)][:, k % kc, bcol],
                            rhs=wi_cs[k // KC][:, k % KC, :],
                            start=False,
                            stop=(k == KT - 1),
                        )
                    if use_bias:
                        nc.gpsimd.tensor_add(pA, pA, bias_sb[:, 0:E])
                    st[jj] = [pAB, None, None, None, None, None]

                def ln_side(jj, side, ps, mv, o):
                    """var -> sqrt -> rstd -> (s-mean)*rstd into o[:, side, :]

                    ps is the bf16 SBUF staging of s (not PSUM): LN runs at
                    2x DVE rate and, crucially, PSUM banks are released by
                    the ACT staging copy, so matmuls never WAR-wait on the
                    DVE/ACT stats chain."""
                    sd = work.tile([P, 1], f32, tag=f"sd{side}")
                    nc.scalar.activation(
                        out=sd, in_=mv[:, side, 1:2],
                        func=mybir.ActivationFunctionType.Sqrt,
                        bias=eps_col, scale=1.0,
                    )
                    rstd = work.tile([P, 1], f32, tag=f"rstd{side}")
                    nc.vector.reciprocal(rstd, sd)
                    dst = o[:, side, :]
                    if use_gb:
                        dst = work.tile([P, E], f32, tag=f"n{side}")
                    nc.vector.tensor_scalar(
                        out=dst, in0=ps,
                        scalar1=mv[:, side, 0:1],
                        scalar2=rstd,
                        op0=sub, op1=mult,
                    )
                    if use_gb:
                        scaled = work.tile([P, E], f32, tag=f"sc{side}")
                        nc.gpsimd.tensor_mul(scaled, dst, lng_sb)
                        nc.gpsimd.tensor_add(o[:, side, :], scaled, lnb_sb)

                def stage2_copy(jj):
                    """s_img -> SBUF (bf16) on ACT.

                    Emitted at the head of each step's ACT queue: this copy
                    feeds the PE transposes AND releases the pAB bank's img
                    half, so it must never sit behind the stats chain.
                    """
                    pAB = st[jj][0]
                    sA = work.tile([P, E], tdt, tag="sA", name=f"sA{jj}")
                    nc.scalar.copy(sA, pAB[:, 0, :])
                    st[jj][1] = sA

                def stage2_rest(jj):
                    """PE-transpose of s_img + img-side LN off the bf16 copy"""
                    sA = st[jj][1]
                    sAv = sA.bitcast(f32) if tdt == mybir.dt.float32r else sA
                    mv = work.tile([P, 2, 2], f32, tag="mv", name=f"mv{jj}")
                    stats = work.tile([P, 6], f32, tag="st0")
                    nc.vector.bn_stats(out=stats, in_=sAv)
                    nc.vector.bn_aggr(out=mv[:, 0, :], in_=stats)
                    pT = psT.tile([P, E], tdt, tag="pT", name=f"pT{jj}")
                    for et in range(2):
                        nc.tensor.transpose(
                            pT[:, et * P : (et + 1) * P],
                            sA[:, et * P : (et + 1) * P],
                            ident,
                        )
                    o = outp.tile([P, 2, E], odt, tag="o", name=f"o{jj}")
                    ln_side(jj, 0, sAv, mv, o)
                    st[jj][2] = mv
                    st[jj][3] = pT
                    st[jj][5] = o

                def stage3(jj):
                    """stage the transposed s_img to SBUF for the pC matmul
                    (ACT: GPSIMD cannot read PSUM on trn2)"""
                    pT = st[jj][3]
                    sT = work.tile([P, E], xdt, tag="sT", name=f"sT{jj}")
                    nc.scalar.copy(sT, pT)
                    st[jj][4] = sT

                def stage4_mm(jj):
                    """tab-attention matmuls into the pAB bank's tab half"""
                    pAB, sA, mv, pT, sT, o = st[jj]
                    pB = pAB[:, 1, :]
                    for et in range(2):
                        nc.tensor.matmul(
                            pB,
                            lhsT=sT[:, et * P : (et + 1) * P],
                            rhs=wct_sb[:, et, :],
                            start=False,
                            stop=(et == 1),
                        )
                    if use_bias:
                        nc.gpsimd.tensor_add(pB, pB, bias_sb[:, E : 2 * E])

                def stage4_rest(jj):
                    """s_tab -> SBUF (releases the bank), LN.

                    Runs one step after stage4_mm so the ACT copy's wait on
                    the pC matmuls is pre-satisfied — nothing on the ACT
                    FIFO may wait on a same-step late producer, or it delays
                    the next step's critical s_img copy."""
                    pAB, sA, mv, pT, sT, o = st[jj]
                    sB = work.tile([P, E], tdt, tag="sB", name=f"sB{jj}")
                    nc.scalar.copy(sB, pAB[:, 1, :])
                    sBv = sB.bitcast(f32) if tdt == mybir.dt.float32r else sB
                    stats = work.tile([P, 6], f32, tag="st1")
                    nc.vector.bn_stats(out=stats, in_=sBv)
                    nc.vector.bn_aggr(out=mv[:, 1, :], in_=stats)
                    ln_side(jj, 1, sBv, mv, o)

                def stage5(jj):
                    """store, one step after the apply completed.

                    The store's dma_start WAITS for its data before
                    generating descriptors, so on the ACT ring it must only
                    be emitted once the apply is already done — otherwise it
                    blocks the next step's critical s_img copy.  (Only SP/ACT
                    have HWDGE rings; SP carries loads.)
                    """
                    o = st[jj][5]
                    nc.scalar.dma_start(out=out_r[:, jj, :], in_=o)

                # per-step emission order tuned per engine FIFO:
                #   PE:   17 MMs(j) | pC(j-3) | transposes(j-1)
                #   ACT:  copy_sA(j-1) first, then sqrts
                #   Pool: sT(j-2) before copy_sB(j-3)
                loaded = set()
                for step in range(NB + 5):
                    if step < NB:
                        s_cur = btiles[step][0]
                        if s_cur not in loaded:
                            loaded.add(s_cur)
                            load_slab(s_cur)
                        stage1(step)
                    if 0 <= step - 1 < NB:
                        stage2_copy(step - 1)
                    if 0 <= step - 3 < NB:
                        stage4_mm(step - 3)
                    if 0 <= step - 1 < NB:
                        stage2_rest(step - 1)
                    if 0 <= step - 2 < NB:
                        stage3(step - 2)
                    if 0 <= step - 4 < NB:
                        stage4_rest(step - 4)
                    if 0 <= step - 5 < NB:
                        stage5(step - 5)

    return nc


def _prep_inputs(inputs: dict, mode: str):
    """Host-side shard + weight folding. Returns (in_maps, use_bias, use_gb)."""
    import ml_dtypes

    f = lambda k: np.asarray(inputs[k], dtype=np.float64)
    Wi, bi = f("Wi"), f("bi")
    Wt, bt = f("Wt"), f("bt")
    Wc_img = f("out_w_img") @ f("in_w_img")[2 * E :]
    bc_img = f("out_w_img") @ f("in_b_img")[2 * E :] + f("out_b_img")
    Wc_tab = f("out_w_tab") @ f("in_w_tab")[2 * E :]
    bc_tab = f("out_w_tab") @ f("in_b_tab")[2 * E :] + f("out_b_tab")

    Wfold_img = Wc_img @ Wt  # (E, TAB_DIM)
    Wt_eff = Wt - Wc_tab @ Wfold_img  # (E, TAB_DIM)
    bias_img = bi + Wc_img @ bt + bc_img
    bias_tab = bt + Wc_tab @ bi + bc_tab
    # pB accumulates s_img @ Wc_tab.T which already includes bias_img
    bias_tab_eff = bias_tab - Wc_tab @ bias_img
    bias = np.concatenate([bias_img, bias_tab_eff]).astype(np.float32)

    lng = np.asarray(inputs["ln_g"], dtype=np.float32)
    lnb = np.asarray(inputs["ln_b"], dtype=np.float32)
    use_bias = bool(np.any(bias != 0.0))
    use_gb = bool(np.any(lng != 1.0) or np.any(lnb != 0.0))

    xdt = ml_dtypes.bfloat16 if mode == "bf16" else np.float32
    wiT = np.ascontiguousarray(Wi.T).astype(xdt)
    wxt = np.concatenate([Wfold_img.T, Wt_eff.T], axis=1).astype(xdt)  # (128, 512)
    wctT = np.ascontiguousarray(Wc_tab.T).astype(xdt)

    xi = np.asarray(inputs["img_embed"], dtype=np.float32)
    xt = np.asarray(inputs["tab_embed"], dtype=np.float32)
    xiT = np.ascontiguousarray(xi.T).astype(xdt)  # (IMG_DIM, B)
    xtT = np.ascontiguousarray(xt.T).astype(xdt)  # (TAB_DIM, B)

    in_maps = []
    for c in range(N_CORES):
        bs = slice(c * B_LOC, (c + 1) * B_LOC)
        m = {
            "xiT": np.ascontiguousarray(xiT[:, bs]),
            "xtT": np.ascontiguousarray(xtT[:, bs]),
            "wiT": wiT,
            "wxt": wxt,
            "wctT": wctT,
        }
        if use_bias:
            m["bias"] = bias
        if use_gb:
            m["lng"] = lng
            m["lnb"] = lnb
        in_maps.append(m)
    return in_maps, use_bias, use_gb


def _kernel_impl(inputs: dict, trace: bool):
    from concourse.bass_utils import run_bass_kernel_spmd

    mode = MM_MODE
    in_maps, use_bias, use_gb = _prep_inputs(inputs, mode)
    key = (use_bias, use_gb, mode)
    if key not in _cache:
        nc = _build_module(use_bias, use_gb, mode)
        # work around this walrus build's 1-wait-per-instruction limit
        orig = nc.to_json_bytes
        nc.to_json_bytes = lambda: _split_multi_waits(orig())
        _cache[key] = nc
    nc = _cache[key]

    try:
        res = run_bass_kernel_spmd(
            nc,
            in_maps,
            core_ids=list(range(N_CORES)),
            trace=trace,
            trace_cores=[0] if trace else None,
        )
    except ModuleNotFoundError:
        # no NTFF profile hook in this container; run without trace
        res = run_bass_kernel_spmd(nc, in_maps, core_ids=list(range(N_CORES)))
    out = np.concatenate([r["out"] for r in res.results], axis=0)
    return np.ascontiguousarray(out.astype(np.float32)), res


def kernel(**inputs) -> np.ndarray:
    out, _ = _kernel_impl(inputs, trace=False)
    return out


def kernel_traced(**inputs):
    return _kernel_impl(inputs, trace=True)
